# revision 1
# baseline (speedup 1.0000x reference)
"""TRN2 Bass kernel for nn_LocalSelfAttn (LN -> packed QKV -> banded attention
(window +-16) -> out-proj -> residual), sharded 8-way over (B, T).

Sharding: 8 cores x 1024 tokens (batch b = cores 4b..4b+3). Each core gets a
halo-padded strip of 1280 tokens (128 halo each side, zero-padded at batch
edges) and computes its 1024 output tokens independently -- no collectives.

Device pipeline per core (all matmuls bf16, fp32 accumulation):
  1. LN (bn_stats/bn_aggr, affine folded into QKV weights on host)
  2. h transposed to [D, tok] via PE transposes
  3. Q,K projection in transposed layout [1024, tok]; V projection in
     token-major layout [tok, 512] shifted by 112 so PV contraction chunks
     are partition-aligned
  4. per 128-query block: S = Q.T^T K.T (8 heads), exp on ACT, mask-mul +
     row-sum fused (tensor_tensor_reduce), normalize, PE-transpose P,
     PV with V token-major, out-proj + rank-1 bias, residual add
"""

import sys

for _p in ("/opt/trn_rl_repo",):
    if _p not in sys.path:
        sys.path.insert(0, _p)

import numpy as np
import ml_dtypes

import concourse.bass as bass
import concourse.tile as tile
from concourse import bacc, mybir
from concourse.bass import ts
from concourse.bass_utils import run_bass_kernel_spmd
from concourse.masks import make_identity

F32 = mybir.dt.float32
BF16 = mybir.dt.bfloat16
AF = mybir.ActivationFunctionType
ALU = mybir.AluOpType

B, T, D, H, BAND = 2, 4096, 512, 8, 16
DH = D // H            # 64
LN_EPS = 1e-5
N_CORES = 8
PC = 1024              # tokens per core
HALO = 128
ST = PC + 2 * HALO     # strip tokens = 1280
NT = ST // 128         # 10 LN tiles
NB = PC // 128         # 8 query blocks
WIN = 160              # key window per 128-query block
VOFF = 112             # v_sb token offset: windows start at 128b+112

_NC_CACHE = None
import os
KBISECT = os.environ.get("KBISECT", "full")


def build_bass():
    nc = bacc.Bacc(None, target_bir_lowering=False)
    xin = nc.declare_dram_parameter("xin", [ST, D], F32, isOutput=False)
    wall = nc.declare_dram_parameter("wall", [4, 128, 3 * D], BF16, isOutput=False)
    wout = nc.declare_dram_parameter("wout", [8, 64, D], BF16, isOutput=False)
    beffqk = nc.declare_dram_parameter("beffqk", [128, 8], F32, isOutput=False)
    bvrow = nc.declare_dram_parameter("bvrow", [1, D], BF16, isOutput=False)
    boutrow = nc.declare_dram_parameter("boutrow", [1, D], BF16, isOutput=False)
    bmask = nc.declare_dram_parameter("bmask", [NB, 128, WIN], BF16, isOutput=False)
    yout = nc.declare_dram_parameter("yout", [PC, D], F32, isOutput=True)

    with tile.TileContext(nc) as tc:
        from contextlib import ExitStack

        with ExitStack() as ctx:
            const = ctx.enter_context(tc.tile_pool(name="const", bufs=1))
            sb = ctx.enter_context(tc.tile_pool(name="sb", bufs=1))
            ln = ctx.enter_context(tc.tile_pool(name="ln", bufs=4))
            cp = ctx.enter_context(tc.tile_pool(name="cp", bufs=3))
            at = ctx.enter_context(tc.tile_pool(name="at", bufs=2))

            # ---- constants ----
            w_sb = const.tile([128, 4, 3 * D], BF16)
            nc.sync.dma_start(out=w_sb, in_=wall.rearrange("c p j -> p c j"))
            wo_sb = const.tile([64, 8, D], BF16)
            nc.sync.dma_start(out=wo_sb, in_=wout.rearrange("c p j -> p c j"))
            beff_sb = const.tile([128, 8], F32)
            nc.sync.dma_start(out=beff_sb, in_=beffqk[:, :])
            bv_sb = const.tile([1, D], BF16)
            nc.sync.dma_start(out=bv_sb, in_=bvrow[:, :])
            bo_sb = const.tile([1, D], BF16)
            nc.sync.dma_start(out=bo_sb, in_=boutrow[:, :])
            bm_sb = const.tile([128, NB, WIN], BF16)
            nc.sync.dma_start(out=bm_sb, in_=bmask.rearrange("b p j -> p b j"))
            ones_sb = const.tile([1, 128], BF16)
            nc.vector.memset(ones_sb, 1.0)
            ident = const.tile([128, 128], BF16)
            make_identity(nc, ident)
            eps_sb = const.tile([128, 1], F32)
            nc.vector.memset(eps_sb, LN_EPS)

            # ---- persistent activations ----
            x_sb = sb.tile([128, NT, D], F32)          # input tiles (also residual)
            ht_sb = sb.tile([128, 4, ST], BF16)        # h^T: [D(4x128), tok]
            qk_sb = sb.tile([128, 8, ST], BF16)        # qk^T: [1024(8x128), tok]
            v_sb = sb.tile([128, 9, D], BF16)          # v token-major, tokens VOFF..VOFF+1152
            qk2_sb = sb.tile([64, 8, ST], BF16)        # odd-head q/k rows shifted to base partition 0

            # ================= Phase A: LN + h^T + QKV =================
            with tc.tile_pool(name="psA", bufs=2, space="PSUM") as psA:
                for t in range(NT):
                    nc.sync.dma_start(out=x_sb[:, t, :], in_=xin[ts(t, 128), :])
                    stats = ln.tile([128, 6], F32)
                    nc.vector.bn_stats(out=stats, in_=x_sb[:, t, :])
                    mv = ln.tile([128, 2], F32)
                    nc.vector.bn_aggr(out=mv, in_=stats)
                    std = ln.tile([128, 1], F32)
                    nc.scalar.activation(out=std, in_=mv[:, 1:2], func=AF.Sqrt, bias=eps_sb)
                    rstd = ln.tile([128, 1], F32)
                    nc.vector.reciprocal(out=rstd, in_=std)
                    nbias = ln.tile([128, 1], F32)
                    nc.vector.tensor_scalar(
                        out=nbias, in0=mv[:, 0:1], scalar1=rstd, scalar2=-1.0,
                        op0=ALU.mult, op1=ALU.mult)
                    hbf = cp.tile([128, D], BF16)
                    nc.scalar.activation(out=hbf, in_=x_sb[:, t, :], func=AF.Identity,
                                         bias=nbias, scale=rstd)
                    tp = psA.tile([128, 4, 128], BF16, tag="tr")
                    for ic in range(4):
                        nc.tensor.transpose(tp[:, ic, :], hbf[:, ts(ic, 128)], ident)
                    nc.vector.tensor_copy(ht_sb[:, :, ts(t, 128)], tp)

                # Q,K projection: out rows j (8 chunks of 128), moving = tokens
                for jc in range(8):
                    qp = psA.tile([128, ST], F32, tag="qk")
                    for s0 in range(0, ST, 512):
                        sn = min(512, ST - s0)
                        for ic in range(4):
                            nc.tensor.matmul(
                                qp[:, s0:s0 + sn],
                                lhsT=w_sb[:, ic, ts(jc, 128)],
                                rhs=ht_sb[:, ic, s0:s0 + sn],
                                start=(ic == 0), stop=(ic == 3))
                    nc.vector.tensor_scalar_add(
                        out=qk_sb[:, jc, :], in0=qp, scalar1=beff_sb[:, jc:jc + 1])

                # V projection token-major, shifted by VOFF
                for vt in range(9):
                    vp = psA.tile([128, D], F32, tag="qk")
                    base = VOFF + 128 * vt
                    for ic in range(4):
                        nc.tensor.matmul(
                            vp, lhsT=ht_sb[:, ic, base:base + 128],
                            rhs=w_sb[:, ic, 2 * D:3 * D],
                            start=(ic == 0), stop=False)
                    nc.tensor.matmul(vp, lhsT=ones_sb, rhs=bv_sb, start=False, stop=True)
                    nc.vector.tensor_copy(v_sb[:, vt, :], vp)

            nc.sync.dma_start(out=qk2_sb, in_=qk_sb[64:128, :, :])

            # ================= Phase B: attention blocks =================
            if KBISECT == "A":
                for b in range(NB):
                    ysb = at.tile([128, D], F32, tag="ysb")
                    nc.vector.tensor_add(out=ysb, in0=x_sb[:, b + 1, :],
                                         in1=qk_sb[:, b, 0:D])
                    nc.vector.tensor_add(out=ysb, in0=ysb, in1=v_sb[:, b, :])
                    nc.sync.dma_start(out=yout[ts(b, 128), :], in_=ysb)
            with (
                tc.tile_pool(name="psS", bufs=3, space="PSUM") as psS,
                tc.tile_pool(name="psB", bufs=1, space="PSUM") as psB,
            ):
                for b in range(NB if KBISECT != "A" else 0):
                    q0 = 128 * (b + 1)
                    k0 = q0 - BAND
                    e_sb = at.tile([128, H, WIN], BF16, tag="e")
                    em_sb = at.tile([128, H, WIN], BF16, tag="em")
                    s_sb = at.tile([128, H], F32, tag="s")
                    si_sb = at.tile([128, H], F32, tag="si")
                    for c in range(4):
                        sp = psS.tile([128, 2, WIN], F32, tag="sc")
                        for hh in range(2):
                            h = 2 * c + hh
                            qc = h // 2
                            src_t = qk_sb if h % 2 == 0 else qk2_sb
                            pp = 0 if h % 2 == 0 else None
                            lhs = (qk_sb[0:64, qc, q0:q0 + 128] if h % 2 == 0
                                   else qk2_sb[:, qc, q0:q0 + 128])
                            rhs = (qk_sb[0:64, 4 + qc, k0:k0 + WIN] if h % 2 == 0
                                   else qk2_sb[:, 4 + qc, k0:k0 + WIN])
                            nc.tensor.matmul(sp[:, hh, :], lhsT=lhs, rhs=rhs,
                                             start=True, stop=True)
                        if KBISECT == "B0":
                            nc.vector.tensor_copy(e_sb[:, 2 * c:2 * c + 2, :], sp)
                        else:
                            nc.scalar.activation(out=e_sb[:, 2 * c:2 * c + 2, :], in_=sp,
                                                 func=AF.Exp)
                    if KBISECT in ("B1a", "B0", "B0q"):
                        ysb = at.tile([128, D], F32, tag="ysb")
                        nc.vector.tensor_add(out=ysb, in0=x_sb[:, b + 1, :],
                                             in1=e_sb[:, 0:4, 0:128])
                        nc.sync.dma_start(out=yout[ts(b, 128), :], in_=ysb)
                        continue
                    for h in range(H):
                        nc.vector.tensor_mul(
                            out=em_sb[:, h, :], in0=e_sb[:, h, :], in1=bm_sb[:, b, :])
                    nc.vector.reduce_sum(out=s_sb, in_=em_sb, axis=mybir.AxisListType.X)
                    nc.vector.reciprocal(out=si_sb, in_=s_sb)
                    for h in range(H):
                        nc.vector.tensor_scalar_mul(
                            out=em_sb[:, h, :], in0=em_sb[:, h, :],
                            scalar1=si_sb[:, h:h + 1])

                    if KBISECT == "B1":
                        ysb = at.tile([128, D], F32, tag="ysb")
                        nc.vector.tensor_add(out=ysb, in0=x_sb[:, b + 1, :],
                                             in1=em_sb[:, 0:4, 0:128])
                        nc.sync.dma_start(out=yout[ts(b, 128), :], in_=ysb)
                        continue
                    pthi = psB.tile([128, H, 128], BF16, tag="pthi")
                    ptlo = psB.tile([32, H, 128], BF16, tag="ptlo")
                    for h in range(H):
                        nc.tensor.transpose(pthi[:, h, :], em_sb[:, h, 0:128], ident)
                        nc.tensor.transpose(ptlo[:, h, :], em_sb[:, h, 128:WIN], ident)
                    pthi_sb = at.tile([128, H, 128], BF16, tag="pthis")
                    ptlo_sb = at.tile([32, H, 128], BF16, tag="ptlos")
                    nc.vector.tensor_copy(pthi_sb, pthi)
                    nc.vector.tensor_copy(ptlo_sb, ptlo)

                    if KBISECT == "B2":
                        ysb = at.tile([128, D], F32, tag="ysb")
                        nc.vector.tensor_add(out=ysb, in0=x_sb[:, b + 1, :],
                                             in1=pthi_sb[:, 0:4, :])
                        nc.vector.tensor_add(out=ysb, in0=ysb, in1=ptlo_sb[0:32, 0:4, :].to_broadcast([128, 4, 128]) if False else ysb)
                        nc.sync.dma_start(out=yout[ts(b, 128), :], in_=ysb)
                        continue
                    pv = psB.tile([64, H, 128], F32, tag="pv")
                    for h in range(H):
                        nc.tensor.matmul(
                            pv[:, h, :],
                            lhsT=v_sb[:, b, ts(h, DH)],
                            rhs=pthi_sb[:, h, :], start=True, stop=False)
                        nc.tensor.matmul(
                            pv[:, h, :],
                            lhsT=v_sb[0:32, b + 1, ts(h, DH)],
                            rhs=ptlo_sb[0:32, h, :], start=False, stop=True)
                    osb = at.tile([64, H, 128], BF16, tag="o")
                    nc.scalar.copy(out=osb, in_=pv)

                    yp = psB.tile([128, D], F32, tag="y")
                    for h in range(H):
                        nc.tensor.matmul(yp, lhsT=osb[:, h, :], rhs=wo_sb[:, h, :],
                                         start=(h == 0), stop=False)
                    nc.tensor.matmul(yp, lhsT=ones_sb, rhs=bo_sb, start=False, stop=True)
                    ysb = at.tile([128, D], F32, tag="ysb")
                    nc.vector.tensor_add(out=ysb, in0=yp, in1=x_sb[:, b + 1, :])
                    nc.sync.dma_start(out=yout[ts(b, 128), :], in_=ysb)
    nc.finalize()
    return nc


def make_in_maps(x, ln_g, ln_b, w_in, b_in, w_out, b_out):
    x = np.asarray(x, np.float32)
    ln_g = np.asarray(ln_g, np.float32)
    ln_b = np.asarray(ln_b, np.float32)
    w_in = np.asarray(w_in, np.float32)
    b_in = np.asarray(b_in, np.float32)
    w_out = np.asarray(w_out, np.float32)
    b_out = np.asarray(b_out, np.float32)

    # fold LN affine + 1/sqrt(dh) into the packed projection
    w_eff = w_in * ln_g[None, :]
    b_eff = b_in + w_in @ ln_b
    sc = np.float32(1.0 / np.sqrt(DH))
    w_eff = w_eff.copy()
    w_eff[:D] *= sc
    b_eff = b_eff.copy()
    b_eff[:D] *= sc

    bf = ml_dtypes.bfloat16
    # [3D, D] -> transpose [D, 3D] -> i-chunks [4, 128, 3D]
    wall = np.ascontiguousarray(w_eff.T.reshape(4, 128, 3 * D)).astype(bf)
    wout = np.ascontiguousarray(w_out.T.reshape(8, 64, D)).astype(bf)
    beffqk = np.ascontiguousarray(b_eff[:2 * D].reshape(8, 128).T).astype(np.float32)
    bvrow = b_eff[2 * D:].reshape(1, D).astype(bf)
    boutrow = b_out.reshape(1, D).astype(bf)

    tt = np.arange(128)[:, None]
    jj = np.arange(WIN)[None, :]
    band = (jj - tt >= 0) & (jj - tt <= 2 * BAND)

    in_maps = []
    for c in range(N_CORES):
        batch = c // 4
        t0 = (c % 4) * PC
        xloc = np.zeros((ST, D), np.float32)
        lo = t0 - HALO
        s0, s1 = max(lo, 0), min(t0 + PC + HALO, T)
        xloc[s0 - lo:s1 - lo] = x[batch, s0:s1]
        bm = np.zeros((NB, 128, WIN), np.float32)
        for b in range(NB):
            gk = t0 + 128 * b - BAND + jj  # global key index of window col j
            bm[b] = band & (gk >= 0) & (gk < T)
        in_maps.append(dict(
            xin=xloc, wall=wall, wout=wout, beffqk=beffqk,
            bvrow=bvrow, boutrow=boutrow, bmask=bm.astype(bf)))
    return in_maps


def kernel_run(inputs, trace=False, trace_kwargs=None):
    global _NC_CACHE
    if _NC_CACHE is None:
        _NC_CACHE = build_bass()
    nc = _NC_CACHE
    in_maps = make_in_maps(**inputs)
    kw = {}
    if trace:
        kw = dict(trace=True, trace_cores=[0], **(trace_kwargs or {}))
    res = run_bass_kernel_spmd(nc, in_maps, list(range(N_CORES)), **kw)
    y = np.stack([res.results[c]["yout"] for c in range(N_CORES)])
    out = y.reshape(B, T, D).astype(np.float32)
    return out, res


def kernel(**inputs):
    out, _ = kernel_run(inputs, trace=False)
    return out


if __name__ == "__main__":
    rng = np.random.default_rng(0)
    ins = dict(
        x=rng.standard_normal((B, T, D)).astype(np.float32),
        ln_g=np.ones(D, np.float32), ln_b=np.zeros(D, np.float32),
        w_in=(rng.standard_normal((3 * D, D)) * 0.02).astype(np.float32),
        b_in=np.zeros(3 * D, np.float32),
        w_out=(rng.standard_normal((D, D)) * 0.02).astype(np.float32),
        b_out=np.zeros(D, np.float32))
    out = kernel(**ins)
    print("ran:", out.shape, out.dtype)

